# revision 6
# baseline (speedup 1.0000x reference)
"""Trainium2 Bass kernel for nn_DiagnosticRIN (B=4, S=2048, V=32000, D=256).

Sharding: the 1024 scan lanes (b, d) go one-per-partition on 8 cores
(core k owns b=k//2, d in [128*(k%2), +128)); per-step state is a [128, 2]
tile (free = real/imag). combined is all-gathered, then each core computes
logits for its 4000-row vocab shard with float32r matmuls.

Numerics: every step op replicates the neuron-compiled reference bit-exactly
(validated on hardware): IEEE division built from exact reciprocal-multiply +
Dekker residual + half-ulp adjust; floor/mod via 2^23 round-trip and mantissa
masking; sin/cos via ACT Sin after the exact >=pi wrap (== device jnp.sin).
"""
import numpy as np
import concourse.bass as bass
from concourse import bacc
import concourse.tile as tile
from concourse import mybir
from concourse.bass_utils import run_bass_kernel_spmd

F32 = mybir.dt.float32
U32 = mybir.dt.uint32
ALU = mybir.AluOpType
AF = mybir.ActivationFunctionType

PHI = np.float32((1.0 + 5.0 ** 0.5) / 2.0)
LUT = 4096
TWO_PI = 2.0 * np.pi
SCALE = float(np.float32(LUT / TWO_PI))
GS = float(np.float32(TWO_PI / LUT))
C23 = float(np.float32(2.0 ** 23))
PI_F = float(np.float32(np.pi))
PI_2 = float(np.float32(np.pi / 2))
TP_F = float(np.float32(TWO_PI))
B_, S_, V_, D_ = 4, 2048, 32000, 256
NCORE = 8
VSH = V_ // NCORE

_tables = None
_last_exec_ns = None


def device_tables():
    global _tables
    if _tables is None:
        import jax.numpy as jnp
        grid = jnp.arange(LUT, dtype=jnp.float32) * (TWO_PI / LUT)
        _tables = (np.asarray(jnp.sin(grid)), np.asarray(jnp.cos(grid)))
    return _tables


def build(S, n_cores):
    nc = bacc.Bacc('TRN2', target_bir_lowering=False, debug=False,
                   num_devices=n_cores)
    lam = nc.dram_tensor("lam", [128, S], F32, kind="ExternalInput").ap()
    lamh = nc.dram_tensor("lamh", [128, S], F32, kind="ExternalInput").ap()
    laml = nc.dram_tensor("laml", [128, S], F32, kind="ExternalInput").ap()
    rlam = nc.dram_tensor("rlam", [128, S], F32, kind="ExternalInput").ap()
    bph = nc.dram_tensor("bph", [128, S], F32, kind="ExternalInput").ap()
    x0 = nc.dram_tensor("x0", [128, 8], F32, kind="ExternalInput").ap()
    projT = nc.dram_tensor("projT", [4, 128, VSH], F32, kind="ExternalInput").ap()
    out = nc.dram_tensor("out", [B_, S, VSH], F32, kind="ExternalOutput").ap()
    cmb = nc.dram_tensor("cmb", [128, 2 * S], F32)
    ag = nc.dram_tensor("ag", [n_cores, 128, 2 * S], F32, addr_space="Shared")

    tphi = [float(np.float32(np.float32(t) * PHI)) for t in range(S)]
    NTTILE = 4 * S // 128          # token tiles (rows of 128 tokens)
    TPB = S // 128                 # token tiles per b

    with tile.TileContext(nc, num_cores=n_cores) as tc:
        with tc.tile_pool(name="c", bufs=1) as cp, \
             tc.tile_pool(name="w", bufs=3) as wp, \
             tc.tile_pool(name="g", bufs=4) as gp, \
             tc.tile_pool(name="ps", bufs=8, space="PSUM") as pp:
            LAM = cp.tile([128, S], F32); nc.sync.dma_start(LAM[:], lam[:])
            LH = cp.tile([128, S], F32); nc.sync.dma_start(LH[:], lamh[:])
            LL = cp.tile([128, S], F32); nc.sync.dma_start(LL[:], laml[:])
            RL = cp.tile([128, S], F32); nc.sync.dma_start(RL[:], rlam[:])
            BP = cp.tile([128, S], F32); nc.sync.dma_start(BP[:], bph[:])
            X0t = cp.tile([128, 8], F32); nc.sync.dma_start(X0t[:], x0[:])
            KT = cp.tile([128, 2 * S], F32)
            SH4 = cp.tile([128, 4], F32)
            nc.vector.memset(SH4[:, 0:2], 0.0)
            nc.vector.memset(SH4[:, 2:4], PI_2)
            BZ = cp.tile([128, 1], F32)
            nc.vector.memset(BZ[:], 0.0)

            X0c = wp.tile([128, 5], F32, tag="X")
            nc.vector.tensor_copy(X0c[:], X0t[:, 0:5])

            def hsum(X, t):
                # h = [cc + (-ss), cs + sc] -> KT cols {t, S+t}
                hv = KT[:, t::S]
                nc.vector.tensor_tensor(hv, X[:, 0:2], X[:, 4:1:-2], ALU.add)

            hsum(X0c, 0)
            Xprev = X0c

            tt = nc.vector.tensor_tensor
            ts = nc.vector.tensor_scalar

            for t in range(1, S):
                h = KT[:, (t - 1)::S]
                LAMc = LAM[:, t:t + 1]
                q0 = wp.tile([128, 2], F32, tag="q0")
                ts(q0[:], h, RL[:, t:t + 1], None, ALU.mult)
                tv = wp.tile([128, 2], F32, tag="tv")
                ts(tv[:], q0[:], 4097.0, None, ALU.mult)
                dv = wp.tile([128, 2], F32, tag="dv")
                tt(dv[:], tv[:], q0[:], ALU.subtract)
                q0h = wp.tile([128, 2], F32, tag="q0h")
                tt(q0h[:], tv[:], dv[:], ALU.subtract)
                q0l = wp.tile([128, 2], F32, tag="q0l")
                tt(q0l[:], q0[:], q0h[:], ALU.subtract)
                p1 = wp.tile([128, 2], F32, tag="p1")
                ts(p1[:], q0[:], LAMc, None, ALU.mult)
                e1 = wp.tile([128, 2], F32, tag="e1")
                ts(e1[:], q0h[:], LH[:, t:t + 1], None, ALU.mult)
                eb = wp.tile([128, 2], F32, tag="eb")
                tt(eb[:], e1[:], p1[:], ALU.subtract)
                e2 = wp.tile([128, 2], F32, tag="e2")
                ts(e2[:], q0l[:], LH[:, t:t + 1], None, ALU.mult)
                eb2 = wp.tile([128, 2], F32, tag="eb2")
                tt(eb2[:], eb[:], e2[:], ALU.add)
                e3 = wp.tile([128, 2], F32, tag="e3")
                ts(e3[:], q0h[:], LL[:, t:t + 1], None, ALU.mult)
                eb3 = wp.tile([128, 2], F32, tag="eb3")
                tt(eb3[:], eb2[:], e3[:], ALU.add)
                e4 = wp.tile([128, 2], F32, tag="e4")
                ts(e4[:], q0l[:], LL[:, t:t + 1], None, ALU.mult)
                eb4 = wp.tile([128, 2], F32, tag="eb4")
                tt(eb4[:], eb3[:], e4[:], ALU.add)
                hp = wp.tile([128, 2], F32, tag="hp")
                tt(hp[:], h, p1[:], ALU.subtract)
                rr = wp.tile([128, 2], F32, tag="rr")
                tt(rr[:], hp[:], eb4[:], ALU.subtract)
                Ex = wp.tile([128, 2], F32, tag="Ex")
                ts(Ex[:].bitcast(U32), q0[:].bitcast(U32), 0x7F800000, None,
                   ALU.bitwise_and)
                Tq = wp.tile([128, 2], F32, tag="Tq")
                ts(Tq[:], Ex[:], float(np.float32(2.0 ** -24)), None, ALU.mult)
                Tt = wp.tile([128, 2], F32, tag="Tt")
                ts(Tt[:], Tq[:], LAMc, None, ALU.mult)
                uu = wp.tile([128, 2], F32, tag="uu")
                ts(uu[:], Tq[:], 2.0, None, ALU.mult)
                a1 = wp.tile([128, 2], F32, tag="a1")
                tt(a1[:], rr[:], Tt[:], ALU.is_gt)
                nT = wp.tile([128, 2], F32, tag="nT")
                ts(nT[:], Tt[:], -1.0, None, ALU.mult)
                a2 = wp.tile([128, 2], F32, tag="a2")
                tt(a2[:], rr[:], nT[:], ALU.is_lt)
                adj = wp.tile([128, 2], F32, tag="adj")
                tt(adj[:], a1[:], a2[:], ALU.subtract)
                st = wp.tile([128, 2], F32, tag="st")
                tt(st[:], adj[:], uu[:], ALU.mult)
                qq = wp.tile([128, 2], F32, tag="qq")
                tt(qq[:], q0[:], st[:], ALU.add)
                # theta = (q + b) + t*phi ; f = theta * SCALE
                th = wp.tile([128, 2], F32, tag="th")
                ts(th[:], qq[:], BP[:, t:t + 1], tphi[t], ALU.add, ALU.add)
                ff = wp.tile([128, 2], F32, tag="ff")
                ts(ff[:], th[:], SCALE, None, ALU.mult)
                # floor
                nn = wp.tile([128, 2], F32, tag="nn")
                ts(nn[:], ff[:], C23, C23, ALU.add, ALU.subtract)
                cmp = wp.tile([128, 2], F32, tag="cmp")
                tt(cmp[:], nn[:], ff[:], ALU.is_gt)
                ii = wp.tile([128, 2], F32, tag="ii")
                tt(ii[:], nn[:], cmp[:], ALU.subtract)
                # mod 4096 via mantissa mask
                t2 = wp.tile([128, 2], F32, tag="t2")
                ts(t2[:], ii[:], C23, None, ALU.add)
                t3 = wp.tile([128, 2], F32, tag="t3")
                ts(t3[:].bitcast(U32), t2[:].bitcast(U32), 0xFFF, 0x4B000000,
                   ALU.bitwise_and, ALU.bitwise_or)
                mm = wp.tile([128, 2], F32, tag="mm")
                ts(mm[:], t3[:], C23, None, ALU.subtract)
                qg = wp.tile([128, 2], F32, tag="qg")
                ts(qg[:], mm[:], GS, None, ALU.mult)
                # y4 = [q_r, q_i, q_r+pi/2, q_i+pi/2]; wrap >= pi -> -2pi
                y4 = wp.tile([128, 4], F32, tag="y4")
                qg4 = qg[:].unsqueeze(1).to_broadcast((128, 2, 2))
                tt(y4[:].rearrange("p (a b) -> p a b", b=2), qg4,
                   SH4[:].rearrange("p (a b) -> p a b", b=2), ALU.add)
                c4 = wp.tile([128, 4], F32, tag="c4")
                ts(c4[:], y4[:], PI_F, None, ALU.is_ge)
                d4 = wp.tile([128, 4], F32, tag="d4")
                ts(d4[:], c4[:], TP_F, None, ALU.mult)
                y4b = wp.tile([128, 4], F32, tag="y4b")
                tt(y4b[:], y4[:], d4[:], ALU.subtract)
                SC = wp.tile([128, 4], F32, tag="SC")
                nc.scalar.activation(SC[:], y4b[:], AF.Sin, bias=BZ[:], scale=1.0)
                X = wp.tile([128, 5], F32, tag="X")
                A = SC[:, 2::-2].unsqueeze(2).to_broadcast((128, 2, 2))
                Bv = SC[:, 3::-2].unsqueeze(1).to_broadcast((128, 2, 2))
                tt(X[:, 0:4].rearrange("p (a b) -> p a b", b=2), A, Bv, ALU.mult)
                ts(X[:, 4:5], X[:, 3:4], -1.0, None, ALU.mult)
                hsum(X, t)
                Xprev = X

            # combined -> DRAM, all-gather
            nc.sync.dma_start(cmb[:], KT[:])
            nc.gpsimd.collective_compute(
                "AllGather", ALU.bypass,
                replica_groups=[list(range(n_cores))],
                ins=[cmb[:]], outs=[ag[:]],
            )

            # GEMM: load projT, round to fp32r
            PJ = []
            for k in range(4):
                pj = cp.tile([128, VSH], F32, tag=f"pj{k}")
                nc.sync.dma_start(pj[:], projT[k])
                pjr = cp.tile([128, VSH], mybir.dt.float32r, tag=f"pjr{k}")
                nc.vector.tensor_copy(pjr[:], pj[:])
                PJ.append(pjr)

            NV = VSH // 500  # 8 tiles of 500
            for j in range(NTTILE):
                b = j // TPB
                t0 = (j % TPB) * 128
                lts = []
                for q in range(4):
                    ri, dh = q // 2, q % 2
                    ci = b * 2 + dh
                    lt = gp.tile([128, 128], F32, tag="lt")
                    nc.sync.dma_start(lt[:], ag[ci, :, ri * S + t0: ri * S + t0 + 128])
                    ltr = gp.tile([128, 128], mybir.dt.float32r, tag="ltr")
                    nc.vector.tensor_copy(ltr[:], lt[:])
                    lts.append(ltr)
                for v in range(NV):
                    ps = pp.tile([128, 500], F32, tag="ps")
                    for k in range(4):
                        nc.tensor.matmul(ps[:], lts[k][:],
                                         PJ[k][:, v * 500:(v + 1) * 500],
                                         start=(k == 0), stop=(k == 3))
                    ev = gp.tile([128, 500], F32, tag="ev")
                    nc.scalar.copy(ev[:], ps[:])
                    nc.sync.dma_start(
                        out[b, t0:t0 + 128, v * 500:(v + 1) * 500], ev[:])
    nc.compile()
    return nc


def host_prep(input_ids, emb_weight, proj_weight, S):
    sin_t, cos_t = device_tables()
    ids = np.asarray(input_ids).astype(np.int32)
    ew = np.ascontiguousarray(np.asarray(emb_weight, dtype=np.float32))
    pw = np.ascontiguousarray(np.asarray(proj_weight, dtype=np.float32))
    maps = []
    for k in range(NCORE):
        b, dh = k // 2, k % 2
        E = ew[ids[b, :S]]                                   # [S, 512]
        w = np.ascontiguousarray(E[:, dh * 128:(dh + 1) * 128].T)   # [128,S]
        bb = np.ascontiguousarray(E[:, 256 + dh * 128: 256 + (dh + 1) * 128].T)
        lam = (np.float32(1.0) + np.abs(w)).astype(np.float32)
        tv = (lam * np.float32(4097.0)).astype(np.float32)
        lh = (tv - (tv - lam).astype(np.float32)).astype(np.float32)
        ll = (lam - lh).astype(np.float32)
        rl = (np.float32(1.0) / lam).astype(np.float32)
        # step 0 (exact, theta_r == theta_i == b_0)
        th0 = bb[:, 0]
        f0 = (th0 * np.float32(SCALE)).astype(np.float32)
        m0 = (np.floor(f0).astype(np.int64) & (LUT - 1)).astype(np.int32)
        s0 = sin_t[m0]; c0 = cos_t[m0]
        cc = (c0 * c0).astype(np.float32); cs = (c0 * s0).astype(np.float32)
        sc = (s0 * c0).astype(np.float32); ss = (s0 * s0).astype(np.float32)
        x0 = np.zeros((128, 8), np.float32)
        x0[:, 0] = cc; x0[:, 1] = cs; x0[:, 2] = sc; x0[:, 3] = ss; x0[:, 4] = -ss
        pj = np.ascontiguousarray(
            pw[k * VSH:(k + 1) * VSH].T.reshape(4, 128, VSH))
        maps.append({"lam": lam, "lamh": lh, "laml": ll, "rlam": rl,
                     "bph": bb, "x0": x0, "projT": pj})
    return maps


_nc_cache = {}


def kernel(input_ids, emb_weight, proj_weight, proj_bias):
    S = np.asarray(input_ids).shape[1]
    maps = host_prep(input_ids, emb_weight, proj_weight, S)
    key = (S, NCORE)
    if key not in _nc_cache:
        _nc_cache[key] = build(S, NCORE)
    nc = _nc_cache[key]
    import time as _time
    _t0 = _time.time()
    r = run_bass_kernel_spmd(nc, maps, list(range(NCORE)))
    global _last_exec_ns
    _last_exec_ns = r.exec_time_ns
    if _last_exec_ns is None:
        _last_exec_ns = int((_time.time() - _t0) * 1e9)
    res = r.results
    logits = np.concatenate(
        [np.asarray(res[k]["out"], dtype=np.float32) for k in range(NCORE)],
        axis=2)
    pb = np.asarray(proj_bias, dtype=np.float32)
    if np.any(pb):
        logits = logits + pb[None, None, :]
    return logits


# revision 7
# speedup vs baseline: 12.8423x; 12.8423x over previous
"""Trainium2 Bass kernel for nn_DiagnosticRIN (B=4, S=2048, V=32000, D=256).

Sharding: the 1024 scan lanes (b, d) go one-per-partition on 8 cores
(core k owns b=k//2, d in [128*(k%2), +128)); per-step state is a [128, 2]
tile (free = real/imag). combined is all-gathered, then each core computes
logits for its 4000-row vocab shard with float32r matmuls.

Numerics: every step op replicates the neuron-compiled reference bit-exactly
(validated on hardware): IEEE division built from exact reciprocal-multiply +
Dekker residual + half-ulp adjust; floor/mod via 2^23 round-trip and mantissa
masking; sin/cos via ACT Sin after the exact >=pi wrap (== device jnp.sin).
"""
import numpy as np
import concourse.bass as bass
from concourse import bacc
import concourse.tile as tile
from concourse import mybir
from concourse.bass_utils import run_bass_kernel_spmd

F32 = mybir.dt.float32
U32 = mybir.dt.uint32
ALU = mybir.AluOpType
AF = mybir.ActivationFunctionType

PHI = np.float32((1.0 + 5.0 ** 0.5) / 2.0)
LUT = 4096
TWO_PI = 2.0 * np.pi
SCALE = float(np.float32(LUT / TWO_PI))
GS = float(np.float32(TWO_PI / LUT))
C23 = float(np.float32(2.0 ** 23))
PI_F = float(np.float32(np.pi))
PI_2 = float(np.float32(np.pi / 2))
TP_F = float(np.float32(TWO_PI))
B_, S_, V_, D_ = 4, 2048, 32000, 256
NCORE = 8
VSH = V_ // NCORE

_tables = None
_last_exec_ns = None


def device_tables():
    global _tables
    if _tables is None:
        import jax.numpy as jnp
        grid = jnp.arange(LUT, dtype=jnp.float32) * (TWO_PI / LUT)
        _tables = (np.asarray(jnp.sin(grid)), np.asarray(jnp.cos(grid)))
    return _tables


def build(S, n_cores):
    nc = bacc.Bacc('TRN2', target_bir_lowering=False, debug=False,
                   num_devices=n_cores)
    lam = nc.dram_tensor("lam", [128, S], F32, kind="ExternalInput").ap()
    lamh = nc.dram_tensor("lamh", [128, S], F32, kind="ExternalInput").ap()
    laml = nc.dram_tensor("laml", [128, S], F32, kind="ExternalInput").ap()
    rlam = nc.dram_tensor("rlam", [128, S], F32, kind="ExternalInput").ap()
    bph = nc.dram_tensor("bph", [128, S], F32, kind="ExternalInput").ap()
    x0 = nc.dram_tensor("x0", [128, 8], F32, kind="ExternalInput").ap()
    projT = nc.dram_tensor("projT", [4, 128, VSH], F32, kind="ExternalInput").ap()
    out = nc.dram_tensor("out", [B_, S, VSH], F32, kind="ExternalOutput").ap()
    cmb = nc.dram_tensor("cmb", [128, 2 * S], F32)
    ag = nc.dram_tensor("ag", [n_cores, 128, 2 * S], F32, addr_space="Shared")

    tphi = [float(np.float32(np.float32(t) * PHI)) for t in range(S)]
    NTTILE = 4 * S // 128          # token tiles (rows of 128 tokens)
    TPB = S // 128                 # token tiles per b

    with tile.TileContext(nc, num_cores=n_cores) as tc:
        with tc.tile_pool(name="c", bufs=1) as cp, \
             tc.tile_pool(name="w", bufs=3) as wp, \
             tc.tile_pool(name="g", bufs=4) as gp, \
             tc.tile_pool(name="ps", bufs=8, space="PSUM") as pp:
            LAM = cp.tile([128, S], F32); nc.sync.dma_start(LAM[:], lam[:])
            LH = cp.tile([128, S], F32); nc.sync.dma_start(LH[:], lamh[:])
            LL = cp.tile([128, S], F32); nc.sync.dma_start(LL[:], laml[:])
            RL = cp.tile([128, S], F32); nc.sync.dma_start(RL[:], rlam[:])
            BP = cp.tile([128, S], F32); nc.sync.dma_start(BP[:], bph[:])
            X0t = cp.tile([128, 8], F32); nc.sync.dma_start(X0t[:], x0[:])
            KT = cp.tile([128, 2 * S], F32)
            SH4 = cp.tile([128, 4], F32)
            nc.vector.memset(SH4[:, 0:2], 0.0)
            nc.vector.memset(SH4[:, 2:4], PI_2)
            BZ = cp.tile([128, 1], F32)
            nc.vector.memset(BZ[:], 0.0)

            X0c = wp.tile([128, 5], F32, tag="X")
            nc.vector.tensor_copy(X0c[:], X0t[:, 0:5])

            def hsum(X, t):
                # h = [cc + (-ss), cs + sc] -> KT cols {t, S+t}
                hv = KT[:, t::S]
                nc.vector.tensor_tensor(hv, X[:, 0:2], X[:, 4:1:-2], ALU.add)

            hsum(X0c, 0)
            Xprev = X0c

            tt = nc.vector.tensor_tensor
            ts = nc.vector.tensor_scalar
            stt = nc.vector.scalar_tensor_tensor

            for t in range(1, S):
                h = KT[:, (t - 1)::S]
                LAMc = LAM[:, t:t + 1]
                q0 = wp.tile([128, 2], F32, tag="q0")
                ts(q0[:], h, RL[:, t:t + 1], None, ALU.mult)
                dv = wp.tile([128, 2], F32, tag="dv")
                stt(dv[:], q0[:], 4097.0, q0[:], ALU.mult, ALU.subtract)
                q0h = wp.tile([128, 2], F32, tag="q0h")
                stt(q0h[:], q0[:], 4097.0, dv[:], ALU.mult, ALU.subtract)
                q0l = wp.tile([128, 2], F32, tag="q0l")
                tt(q0l[:], q0[:], q0h[:], ALU.subtract)
                p1 = wp.tile([128, 2], F32, tag="p1")
                ts(p1[:], q0[:], LAMc, None, ALU.mult)
                eb = wp.tile([128, 2], F32, tag="eb")
                stt(eb[:], q0h[:], LH[:, t:t + 1], p1[:], ALU.mult, ALU.subtract)
                eb2 = wp.tile([128, 2], F32, tag="eb2")
                stt(eb2[:], q0l[:], LH[:, t:t + 1], eb[:], ALU.mult, ALU.add)
                eb3 = wp.tile([128, 2], F32, tag="eb3")
                stt(eb3[:], q0h[:], LL[:, t:t + 1], eb2[:], ALU.mult, ALU.add)
                eb4 = wp.tile([128, 2], F32, tag="eb4")
                stt(eb4[:], q0l[:], LL[:, t:t + 1], eb3[:], ALU.mult, ALU.add)
                hp = wp.tile([128, 2], F32, tag="hp")
                stt(hp[:], p1[:], -1.0, h, ALU.mult, ALU.add)
                rr = wp.tile([128, 2], F32, tag="rr")
                stt(rr[:], eb4[:], -1.0, hp[:], ALU.mult, ALU.add)
                Ex = wp.tile([128, 2], F32, tag="Ex")
                ts(Ex[:].bitcast(U32), q0[:].bitcast(U32), 0x7F800000, None,
                   ALU.bitwise_and)
                Tt = wp.tile([128, 2], F32, tag="Tt")
                ts(Tt[:], Ex[:], float(np.float32(2.0 ** -24)), LAMc,
                   ALU.mult, ALU.mult)
                uu = wp.tile([128, 2], F32, tag="uu")
                ts(uu[:], Ex[:], float(np.float32(2.0 ** -24)), 2.0,
                   ALU.mult, ALU.mult)
                a1 = wp.tile([128, 2], F32, tag="a1")
                tt(a1[:], rr[:], Tt[:], ALU.is_gt)
                a2 = wp.tile([128, 2], F32, tag="a2")
                stt(a2[:], Tt[:], -1.0, rr[:], ALU.mult, ALU.is_gt)
                adj = wp.tile([128, 2], F32, tag="adj")
                stt(adj[:], a2[:], -1.0, a1[:], ALU.mult, ALU.add)
                st = wp.tile([128, 2], F32, tag="st")
                tt(st[:], adj[:], uu[:], ALU.mult)
                qq = wp.tile([128, 2], F32, tag="qq")
                tt(qq[:], q0[:], st[:], ALU.add)
                # theta = (q + b) + t*phi ; f = theta * SCALE
                th = wp.tile([128, 2], F32, tag="th")
                ts(th[:], qq[:], BP[:, t:t + 1], tphi[t], ALU.add, ALU.add)
                ff = wp.tile([128, 2], F32, tag="ff")
                ts(ff[:], th[:], SCALE, None, ALU.mult)
                # floor
                nn = wp.tile([128, 2], F32, tag="nn")
                ts(nn[:], ff[:], C23, C23, ALU.add, ALU.subtract)
                cmp = wp.tile([128, 2], F32, tag="cmp")
                tt(cmp[:], nn[:], ff[:], ALU.is_gt)
                # ii = nn - cmp ; t2 = ii + 2^23  => t2 = (cmp*-1 + nn) + 2^23
                ii = wp.tile([128, 2], F32, tag="ii")
                stt(ii[:], cmp[:], -1.0, nn[:], ALU.mult, ALU.add)
                t2 = wp.tile([128, 2], F32, tag="t2")
                ts(t2[:], ii[:], C23, None, ALU.add)
                t3 = wp.tile([128, 2], F32, tag="t3")
                ts(t3[:].bitcast(U32), t2[:].bitcast(U32), 0xFFF, 0x4B000000,
                   ALU.bitwise_and, ALU.bitwise_or)
                qg = wp.tile([128, 2], F32, tag="qg")
                ts(qg[:], t3[:], C23, GS, ALU.subtract, ALU.mult)
                # y4 = [q_r, q_i, q_r+pi/2, q_i+pi/2]; wrap >= pi -> -2pi
                y4 = wp.tile([128, 4], F32, tag="y4")
                qg4 = qg[:].unsqueeze(1).to_broadcast((128, 2, 2))
                tt(y4[:].rearrange("p (a b) -> p a b", b=2), qg4,
                   SH4[:].rearrange("p (a b) -> p a b", b=2), ALU.add)
                c4 = wp.tile([128, 4], F32, tag="c4")
                ts(c4[:], y4[:], PI_F, None, ALU.is_ge)
                d4 = wp.tile([128, 4], F32, tag="d4")
                ts(d4[:], c4[:], TP_F, None, ALU.mult)
                y4b = wp.tile([128, 4], F32, tag="y4b")
                tt(y4b[:], y4[:], d4[:], ALU.subtract)
                SC = wp.tile([128, 4], F32, tag="SC")
                nc.scalar.activation(SC[:], y4b[:], AF.Sin, bias=BZ[:], scale=1.0)
                X = wp.tile([128, 5], F32, tag="X")
                A = SC[:, 2::-2].unsqueeze(2).to_broadcast((128, 2, 2))
                Bv = SC[:, 3::-2].unsqueeze(1).to_broadcast((128, 2, 2))
                tt(X[:, 0:4].rearrange("p (a b) -> p a b", b=2), A, Bv, ALU.mult)
                ts(X[:, 4:5], X[:, 3:4], -1.0, None, ALU.mult)
                hsum(X, t)
                Xprev = X

            # combined -> DRAM, all-gather
            nc.sync.dma_start(cmb[:], KT[:])
            nc.gpsimd.collective_compute(
                "AllGather", ALU.bypass,
                replica_groups=[list(range(n_cores))],
                ins=[cmb[:]], outs=[ag[:]],
            )

            # GEMM: load projT, round to fp32r
            PJ = []
            for k in range(4):
                pj = cp.tile([128, VSH], F32, tag=f"pj{k}")
                nc.sync.dma_start(pj[:], projT[k])
                pjr = cp.tile([128, VSH], mybir.dt.float32r, tag=f"pjr{k}")
                nc.vector.tensor_copy(pjr[:], pj[:])
                PJ.append(pjr)

            NV = VSH // 500  # 8 tiles of 500
            for j in range(NTTILE):
                b = j // TPB
                t0 = (j % TPB) * 128
                lts = []
                for q in range(4):
                    ri, dh = q // 2, q % 2
                    ci = b * 2 + dh
                    lt = gp.tile([128, 128], F32, tag="lt")
                    nc.sync.dma_start(lt[:], ag[ci, :, ri * S + t0: ri * S + t0 + 128])
                    ltr = gp.tile([128, 128], mybir.dt.float32r, tag="ltr")
                    nc.vector.tensor_copy(ltr[:], lt[:])
                    lts.append(ltr)
                for v in range(NV):
                    ps = pp.tile([128, 500], F32, tag="ps")
                    for k in range(4):
                        nc.tensor.matmul(ps[:], lts[k][:],
                                         PJ[k][:, v * 500:(v + 1) * 500],
                                         start=(k == 0), stop=(k == 3))
                    ev = gp.tile([128, 500], F32, tag="ev")
                    nc.scalar.copy(ev[:], ps[:])
                    nc.sync.dma_start(
                        out[b, t0:t0 + 128, v * 500:(v + 1) * 500], ev[:])
    nc.compile()
    return nc


def host_prep(input_ids, emb_weight, proj_weight, S):
    sin_t, cos_t = device_tables()
    ids = np.asarray(input_ids).astype(np.int32)
    ew = np.ascontiguousarray(np.asarray(emb_weight, dtype=np.float32))
    pw = np.ascontiguousarray(np.asarray(proj_weight, dtype=np.float32))
    maps = []
    for k in range(NCORE):
        b, dh = k // 2, k % 2
        E = ew[ids[b, :S]]                                   # [S, 512]
        w = np.ascontiguousarray(E[:, dh * 128:(dh + 1) * 128].T)   # [128,S]
        bb = np.ascontiguousarray(E[:, 256 + dh * 128: 256 + (dh + 1) * 128].T)
        lam = (np.float32(1.0) + np.abs(w)).astype(np.float32)
        tv = (lam * np.float32(4097.0)).astype(np.float32)
        lh = (tv - (tv - lam).astype(np.float32)).astype(np.float32)
        ll = (lam - lh).astype(np.float32)
        rl = (np.float32(1.0) / lam).astype(np.float32)
        # step 0 (exact, theta_r == theta_i == b_0)
        th0 = bb[:, 0]
        f0 = (th0 * np.float32(SCALE)).astype(np.float32)
        m0 = (np.floor(f0).astype(np.int64) & (LUT - 1)).astype(np.int32)
        s0 = sin_t[m0]; c0 = cos_t[m0]
        cc = (c0 * c0).astype(np.float32); cs = (c0 * s0).astype(np.float32)
        sc = (s0 * c0).astype(np.float32); ss = (s0 * s0).astype(np.float32)
        x0 = np.zeros((128, 8), np.float32)
        x0[:, 0] = cc; x0[:, 1] = cs; x0[:, 2] = sc; x0[:, 3] = ss; x0[:, 4] = -ss
        pj = np.ascontiguousarray(
            pw[k * VSH:(k + 1) * VSH].T.reshape(4, 128, VSH))
        maps.append({"lam": lam, "lamh": lh, "laml": ll, "rlam": rl,
                     "bph": bb, "x0": x0, "projT": pj})
    return maps


_nc_cache = {}


def kernel(input_ids, emb_weight, proj_weight, proj_bias):
    S = np.asarray(input_ids).shape[1]
    maps = host_prep(input_ids, emb_weight, proj_weight, S)
    key = (S, NCORE)
    if key not in _nc_cache:
        _nc_cache[key] = build(S, NCORE)
    nc = _nc_cache[key]
    import time as _time
    _t0 = _time.time()
    r = run_bass_kernel_spmd(nc, maps, list(range(NCORE)))
    global _last_exec_ns
    _last_exec_ns = r.exec_time_ns
    if _last_exec_ns is None:
        _last_exec_ns = int((_time.time() - _t0) * 1e9)
    res = r.results
    logits = np.concatenate(
        [np.asarray(res[k]["out"], dtype=np.float32) for k in range(NCORE)],
        axis=2)
    pb = np.asarray(proj_bias, dtype=np.float32)
    if np.any(pb):
        logits = logits + pb[None, None, :]
    return logits
